# revision 17
# baseline (speedup 1.0000x reference)
"""DRQConv2d (dual-region quantized conv) Trainium2 kernel, v4.

Reference semantics:
  mask  = upsample8(avgpool8(x) >= 0.05)             per (b, c)
  xh    = where(mask, x, 1e-5);  xl = where(mask, 1e-5, x)
  qh    = clip(round(xh/sh), 0, 255) * sh            (uint8 fake-quant)
  ql    = clip(round(xl/sl), 0, 15) * sl             (uint4 fake-quant)
  qwh   = per-oc quant of w_high to +-127,  qwl = per-oc quant of w_low to +-7
  y     = conv3x3(qh, qwh) + conv3x3(ql, qwl)        (pad 1)

v4 design (v3 112us -> target ~85us):
  * Acts stored in zero-padded [58 x 58] planes so every tap runs the full
    8x56 output region: no border fixups, no mini matmuls.
    14 matmuls/chunk: 9 bf16 high taps + 4 fp8 DoubleRow low tap-pairs
    ((0,k)|(2,k) vertical, (1,0)|(1,2) horizontal) + 1 fp8 single (1,1).
  * High weights pre-scaled by ratio_oc = svh/svl in bf16 so both convs
    share one PSUM bank and a single sv_l evacuation scale.
    Low weights are exact ints in fp8 e4m3.
  * No explicit integer rounding: acts are bf16(relu(x/sh)) -> min/mask;
    the bf16 grid (exact ints to 256) is the high quantizer and the fp8
    RTN is the low quantizer (within the 2e-2 tolerance; TRN fp8e4 is
    IEEE e4m3, max normal 240).
  * Low-path act plane derived from the high-path relu for free:
    ql = min(t_h, 15*sl/sh) * (mask ? 0 : sh/sl)  -- the scale ratio is
    folded into the STT clip constant and the mask values.
  * Mask pipeline: DVE tensor_reduce (8-col then 8-row sums), GPSIMD
    threshold + mask-value TS + two small broadcast expansions.
  * One big 4D STT per path per image quantizes the whole padded plane.
  * Evacuation: ACT per-oc scale by svl, then DMA out.

Sharding: data-parallel over batch.  32 images -> 4 per core on 8 cores,
weights replicated; outputs concatenated on host.  No collectives.
"""

import numpy as np
import ml_dtypes

P = 128            # channels (both in and out) == partitions
B_TOTAL = 32
N_CORES = 8
BPC = B_TOTAL // N_CORES   # images per core
H = W = 56
NPIX = H * W       # 3136
NTAPS = 9
RPC = 8                       # output rows per chunk
NCHUNK = H // RPC             # 7
NFREE = RPC * W               # 448 psum columns per chunk
PW = W + 2                    # padded row length (58)
PH = H + 2
PADPIX = PH * PW              # 3364
POOL_K = 8
THRESH = 0.05
WARM_MMS = 16
# low-conv DR tap pairs (j0, j1) and the leftover single tap
LOW_PAIRS = (((0, 0), (2, 0)), ((0, 1), (2, 1)), ((0, 2), (2, 2)),
             ((1, 0), (1, 2)))
LOW_SINGLE = (1, 1)


# ---------------------------------------------------------------- host side

def _host_weight_prep(w, n):
    """Quantize per-oc exactly like the reference (fp32 divide + round-half-
    even + clip).  Returns integer weights [oc, ic, 9] (as fp32) and the
    per-oc weight scale s = absmax/n (fp32)."""
    w = np.asarray(w, dtype=np.float32).reshape(P, P, NTAPS)
    absmax = np.abs(w.reshape(P, -1)).max(axis=1).astype(np.float32)
    s = (absmax / np.float32(n)).astype(np.float32)
    ratio = w / s[:, None, None]          # fp32, like the reference
    wint = np.clip(np.round(ratio), -n, n).astype(np.float32)
    return wint, s


def _prep_inputs(w_high, w_low, act_scale_high, act_scale_low):
    sh = np.float64(np.float32(act_scale_high))
    sl = np.float64(np.float32(act_scale_low))
    inv_sh = float(np.float32(1.0 / sh))
    clip_lo = float(np.float32(15.0 * sl / sh))   # low clip in t_h units
    mlo = float(np.float32(sh / sl))              # low mask value

    wih, s_h = _host_weight_prep(w_high, 127.0)
    wil, s_l = _host_weight_prep(w_low, 7.0)

    bf16 = ml_dtypes.bfloat16
    e4 = ml_dtypes.float8_e4m3

    sv_h64 = sh * s_h.astype(np.float64)
    sv_l64 = sl * s_l.astype(np.float64)
    ratio = (sv_h64 / sv_l64)[None, None, :]      # per-oc

    # high: [ic, tap, oc] bf16, pre-scaled by svh/svl
    qwt_h = np.ascontiguousarray(
        (wih.transpose(1, 2, 0).astype(np.float64) * ratio)
        .astype(bf16)).reshape(P, NTAPS * P)
    # low DR pairs: [ic, pair, j, oc] fp8 (exact small ints)
    wil_t = wil.transpose(1, 2, 0)                # [ic, tap, oc]
    pairs = np.stack([
        np.stack([wil_t[:, a[0] * 3 + a[1], :], wil_t[:, b[0] * 3 + b[1], :]],
                 axis=1)
        for a, b in LOW_PAIRS], axis=1)           # [ic, pair, j, oc]
    qwt_l_p = np.ascontiguousarray(pairs.astype(e4)).reshape(P, 8 * P)
    # low single: [ic, oc] fp8
    ks, kws = LOW_SINGLE
    qwt_l_s = np.ascontiguousarray(
        wil_t[:, ks * 3 + kws, :].astype(e4)).reshape(P, P)

    return {
        "qwt_h": qwt_h,
        "qwt_l_p": qwt_l_p,
        "qwt_l_s": qwt_l_s,
        "sv_l": sv_l64.astype(np.float32).reshape(P, 1),
    }, inv_sh, clip_lo, mlo


# ---------------------------------------------------------------- device side

def build_program(nc, tc, aps, inv_sh, clip_lo, mlo, bpc=BPC):
    import concourse.mybir as mybir
    from concourse.alu_op_type import AluOpType as op

    f32 = mybir.dt.float32
    bf16 = mybir.dt.bfloat16
    fp8 = mybir.dt.float8e4
    DR = mybir.MatmulPerfMode.DoubleRow
    AX = mybir.AxisListType.X

    x_d, y_d = aps["x"], aps["y"]
    sum_thresh = float(np.float32(THRESH) * POOL_K * POOL_K)  # exact pow2 scale

    with (
        tc.tile_pool(name="consts", bufs=1) as consts,
        tc.tile_pool(name="xs", bufs=4) as xs_pool,
        tc.tile_pool(name="ts", bufs=3) as ts_pool,
        tc.tile_pool(name="qs", bufs=3) as qs_pool,
        tc.tile_pool(name="mk", bufs=2) as mk_pool,
        tc.tile_pool(name="ev", bufs=4) as ev_pool,
        tc.tile_pool(name="cps", bufs=8, space="PSUM") as cps,
    ):
        # ---- weights / scales (host-prepped, DMA only)
        qwt_h = consts.tile([P, NTAPS * P], bf16, tag="qwt_h")
        qwt_l_p = consts.tile([P, 8 * P], fp8, tag="qwt_l_p")
        qwt_l_s = consts.tile([P, P], fp8, tag="qwt_l_s")
        sv_l = consts.tile([P, 1], f32, tag="sv_l")

        # ---- PE warm-up: no data deps; runs while DMAs stream in.
        warm_l = consts.tile([P, P], bf16, tag="warm_l")
        warm_r = consts.tile([P, NFREE], bf16, tag="warm_r")
        nc.gpsimd.memset(warm_l[:], 0.0)
        nc.gpsimd.memset(warm_r[:], 0.0)
        warm_ps = cps.tile([P, NFREE], f32, tag="ps", name="warm")
        for i in range(WARM_MMS):
            nc.tensor.matmul(
                warm_ps[:], warm_l[:], warm_r[:],
                start=(i == 0), stop=(i == WARM_MMS - 1),
            )

        # ---- input DMAs; order sets HBM arrival priority for the head
        xts = {}
        for b in range(bpc):
            xts[b] = xs_pool.tile([P, NPIX], f32, tag="xt", name=f"xt{b}")

        def band_dma(b, row0, rows):
            nc.sync.dma_start(
                out=xts[b][:, row0 * W:(row0 + rows) * W],
                in_=x_d[b][:, row0 * W:(row0 + rows) * W],
            )

        nc.sync.dma_start(out=qwt_h[:], in_=aps["qwt_h"])
        band_dma(0, 0, 8)
        band_dma(0, 8, 8)
        nc.sync.dma_start(out=qwt_l_p[:], in_=aps["qwt_l_p"])
        nc.sync.dma_start(out=qwt_l_s[:], in_=aps["qwt_l_s"])
        nc.sync.dma_start(out=sv_l[:], in_=aps["sv_l"])
        band_dma(0, 16, 16)
        band_dma(0, 32, 24)
        for b in range(1, bpc):
            band_dma(b, 0, 24)
            band_dma(b, 24, 32)

        # padded act planes; pad cells are zeroed once per physical buffer
        qh_t, ql_t = {}, {}
        for b in range(bpc):
            qh_t[b] = qs_pool.tile([P, PADPIX], bf16, tag="qh", name=f"qh{b}")
            ql_t[b] = qs_pool.tile([P, PADPIX], fp8, tag="ql", name=f"ql{b}")

        def pad_memsets(b):
            """Zero only the padding ring (rows 0/57, cols 0/57)."""
            for q in (qh_t[b], ql_t[b]):
                q3 = q[:].rearrange("p (r c) -> p r c", c=PW)
                nc.gpsimd.memset(q3[:, 0:PH:PH - 1, :], 0.0)
                nc.gpsimd.memset(q3[:, 1:PH - 1, 0:PW:PW - 1], 0.0)

        def image_tiles(b):
            t = ts_pool.tile([P, NPIX], bf16, tag="t", name=f"t{b}")
            p1 = mk_pool.tile([P, 392], f32, tag="p1", name=f"p1_{b}")
            u1 = mk_pool.tile([P, 196], f32, tag="u1", name=f"u1_{b}")
            u2 = mk_pool.tile([P, 98], f32, tag="u2", name=f"u2_{b}")
            s2 = mk_pool.tile([P, 49], f32, tag="s2", name=f"s2_{b}")
            mlw = mk_pool.tile([P, 49], f32, tag="mlw", name=f"mlw_{b}")
            mw_h = mk_pool.tile([P, 392], bf16, tag="mw_h", name=f"mwh_{b}")
            mw_l = mk_pool.tile([P, 392], bf16, tag="mw_l", name=f"mwl_{b}")
            return t, p1, u1, u2, s2, mlw, mw_h, mw_l

        prep = {}

        def prep_bands(b, hb0, nhb):
            """Prep pool-rows [hb0, hb0+nhb): relu, pool, mask, quantize.
            The small mask-chain ops run on DVE for image 0 (short critical
            path at the head) and on GPSIMD for the rest (keeps DVE free)."""
            t, p1, u1, u2, s2, mlw, mw_h, mw_l = prep[b]
            g = nc.vector if b == 0 else nc.gpsimd
            xt = xts[b]
            r0, rows = hb0 * POOL_K, nhb * POOL_K

            # ACT: t = bf16(relu(x * inv_sh))
            nc.scalar.activation(
                t[:, r0 * W:(r0 + rows) * W], xt[:, r0 * W:(r0 + rows) * W],
                mybir.ActivationFunctionType.Relu, scale=inv_sh,
            )
            # DVE: pool 8 cols: [P, rows*7, 8] -> [P, rows*7]  (row-major
            # (r, wb) fuses to a stride-8 dim).  For image 0 split per hb
            # so each reduce waits only on its own 8-row DMA band.
            if b == 0:
                for hb in range(hb0, hb0 + nhb):
                    nc.vector.tensor_reduce(
                        p1[:, hb * 56:(hb + 1) * 56],
                        xt[:, hb * POOL_K * W:(hb + 1) * POOL_K * W]
                        .rearrange("p (k c) -> p k c", c=POOL_K),
                        AX, op.add,
                    )
            else:
                nc.vector.tensor_reduce(
                    p1[:, r0 * 7:(r0 + rows) * 7],
                    xt[:, r0 * W:(r0 + rows) * W].rearrange(
                        "p (k c) -> p k c", c=POOL_K),
                    AX, op.add,
                )
            # pool 8 rows: per-hb DVE reduce for image 0 (short critical
            # path); 3-round pairwise GPSIMD tree for the rest
            if b == 0:
                for hb in range(hb0, hb0 + nhb):
                    nc.vector.tensor_reduce(
                        s2[:, hb * 7:(hb + 1) * 7],
                        p1[:, hb * 56:(hb + 1) * 56].rearrange(
                            "p (r wb) -> p wb r", r=POOL_K),
                        AX, op.add,
                    )
            else:
                sr0, scount = r0, rows
                for src, dst in ((p1, u1), (u1, u2), (u2, s2)):
                    vin = src[:, sr0 * 7:(sr0 + scount) * 7].rearrange(
                        "p (r wb) -> p r wb", wb=7)
                    g.tensor_tensor(
                        out=dst[:, (sr0 // 2) * 7:((sr0 + scount) // 2) * 7]
                        .rearrange("p (r wb) -> p r wb", wb=7),
                        in0=vin[:, 0:scount:2, :], in1=vin[:, 1:scount:2, :],
                        op=op.add)
                    sr0, scount = sr0 // 2, scount // 2
            # threshold -> {0,1}; low mask value {mlo, 0}
            g.tensor_scalar(
                s2[:, hb0 * 7:(hb0 + nhb) * 7],
                s2[:, hb0 * 7:(hb0 + nhb) * 7],
                sum_thresh, None, op0=op.is_ge)
            g.tensor_scalar(
                mlw[:, hb0 * 7:(hb0 + nhb) * 7],
                s2[:, hb0 * 7:(hb0 + nhb) * 7],
                -mlo, mlo, op0=op.mult, op1=op.add)
            # DVE: expand wb -> 8 cols: [P, nhb*56] bf16 row patterns
            for src, dst in ((s2, mw_h), (mlw, mw_l)):
                nc.vector.tensor_copy(
                    out=dst[:, hb0 * 56:(hb0 + nhb) * 56].rearrange(
                        "p (wb c) -> p wb c", c=POOL_K),
                    in_=src[:, hb0 * 7:(hb0 + nhb) * 7].unsqueeze(2)
                    .broadcast_to((P, nhb * 7, POOL_K)),
                )
            # DVE: per-hb STT: clip + mask (8-row broadcast) -> padded plane
            t3 = t[:].rearrange("p (r c) -> p r c", c=W)
            qh3 = qh_t[b][:].rearrange("p (r c) -> p r c", c=PW)
            ql3 = ql_t[b][:].rearrange("p (r c) -> p r c", c=PW)
            for hb in range(hb0, hb0 + nhb):
                for q3, clip, mw in ((qh3, 255.0, mw_h),
                                     (ql3, clip_lo, mw_l)):
                    in1 = mw[:, hb * 56:(hb + 1) * 56].unsqueeze(1)
                    in1 = in1.broadcast_to((P, POOL_K, W))
                    nc.vector.scalar_tensor_tensor(
                        out=q3[:, 1 + hb * POOL_K:1 + (hb + 1) * POOL_K,
                               1:1 + W],
                        in0=t3[:, hb * POOL_K:(hb + 1) * POOL_K, :],
                        scalar=clip, in1=in1, op0=op.min, op1=op.mult,
                    )

        def conv_chunk(b, c):
            """9 bf16 high taps + 4 DR low pairs + 1 low single + evac."""
            qh3 = qh_t[b][:].rearrange("p (r c) -> p r c", c=PW)
            ql3 = ql_t[b][:].rearrange("p (r c) -> p r c", c=PW)
            r0 = c * RPC
            ps = cps.tile([P, NFREE], f32, tag="ps", name=f"ps{b}_{c}")
            for i, (kh, kw) in enumerate(
                    (kh, kw) for kh in range(3) for kw in range(3)):
                nc.tensor.matmul(
                    ps[:],
                    qwt_h[:, (kh * 3 + kw) * P:(kh * 3 + kw + 1) * P],
                    qh3[:, r0 + kh:r0 + kh + RPC, kw:kw + W],
                    start=(i == 0), stop=False,
                )
            for pi, (ta, tb) in enumerate(LOW_PAIRS):
                off_a = (r0 + ta[0]) * PW + ta[1]
                off_b = (r0 + tb[0]) * PW + tb[1]
                rhs = ql3[:, r0 + ta[0]:r0 + ta[0] + RPC, ta[1]:ta[1] + W]
                rhs = rhs.unsqueeze(1).broadcast_to((P, 2, RPC, W))
                rhs.ap = mybir.VecI64Pair(
                    [[PADPIX, P], [off_b - off_a, 2], [PW, RPC], [1, W]])
                nc.tensor.matmul(
                    ps[:],
                    qwt_l_p[:, pi * 2 * P:(pi + 1) * 2 * P]
                    .rearrange("p (j m) -> p j m", j=2),
                    rhs,
                    start=False, stop=False,
                    perf_mode=DR,
                )
            ks, kws = LOW_SINGLE
            nc.tensor.matmul(
                ps[:], qwt_l_s[:],
                ql3[:, r0 + ks:r0 + ks + RPC, kws:kws + W],
                start=False, stop=True,
            )
            acc = ev_pool.tile([P, NFREE], f32, tag="acc")
            nc.scalar.mul(acc[:], ps[:], sv_l[:, 0:1])
            nc.sync.dma_start(
                out=y_d[b][:, r0 * W:(r0 + RPC) * W], in_=acc[:],
            )

        # ---------------- schedule ----------------
        # image 0 in 3 bands to cut head latency
        pad_memsets(0)
        prep[0] = image_tiles(0)
        prep_bands(0, 0, 2)
        prep_bands(0, 2, 2)
        prep_bands(0, 4, 3)
        conv_chunk(0, 0)
        conv_chunk(0, 1)
        conv_chunk(0, 2)
        # image 1 prep next so in-order queues stay ahead of conv evacs
        if bpc > 1:
            pad_memsets(1)
            if bpc > 2:
                pad_memsets(2)
            prep[1] = image_tiles(1)
            prep_bands(1, 0, 7)
        for c in range(3, NCHUNK):
            conv_chunk(0, c)

        for b in range(1, bpc):
            nb = b + 1
            for c in range(NCHUNK):
                if nb < bpc and c == 1:
                    prep[nb] = image_tiles(nb)
                    prep_bands(nb, 0, 7)
                conv_chunk(b, c)


def make_bass(inv_sh, clip_lo, mlo, bpc=BPC):
    import concourse.bacc as bacc
    import concourse.mybir as mybir
    from concourse.tile import TileContext

    f32 = mybir.dt.float32
    bf16 = mybir.dt.bfloat16
    fp8 = mybir.dt.float8e4
    nc = bacc.Bacc("TRN2", debug=False)
    x = nc.dram_tensor("x", [bpc, P, NPIX], f32, kind="ExternalInput")
    qwh = nc.dram_tensor("qwt_h", [P, NTAPS * P], bf16, kind="ExternalInput")
    qwlp = nc.dram_tensor("qwt_l_p", [P, 8 * P], fp8, kind="ExternalInput")
    qwls = nc.dram_tensor("qwt_l_s", [P, P], fp8, kind="ExternalInput")
    svl = nc.dram_tensor("sv_l", [P, 1], f32, kind="ExternalInput")
    y = nc.dram_tensor("y", [bpc, P, NPIX], f32, kind="ExternalOutput")
    aps = {
        "x": x.ap(), "y": y.ap(),
        "qwt_h": qwh.ap(), "qwt_l_p": qwlp.ap(), "qwt_l_s": qwls.ap(),
        "sv_l": svl.ap(),
    }
    with TileContext(nc) as tc:
        build_program(nc, tc, aps, inv_sh, clip_lo, mlo, bpc=bpc)
    nc.compile()
    return nc


def _run(x, w_high, w_low, act_scale_high, act_scale_low, trace=False, **kw):
    from concourse import bass_utils

    x = np.ascontiguousarray(np.asarray(x, dtype=np.float32))
    w_high = np.asarray(w_high, dtype=np.float32)
    w_low = np.asarray(w_low, dtype=np.float32)

    wmap, inv_sh, clip_lo, mlo = _prep_inputs(
        w_high, w_low, act_scale_high, act_scale_low)
    nc = make_bass(inv_sh, clip_lo, mlo)

    in_maps = []
    for core in range(N_CORES):
        xs = x[core * BPC:(core + 1) * BPC].reshape(BPC, P, NPIX)
        m = {"x": np.ascontiguousarray(xs)}
        m.update(wmap)
        in_maps.append(m)
    res = bass_utils.run_bass_kernel_spmd(
        nc, in_maps, core_ids=list(range(N_CORES)), trace=trace, **kw
    )
    y = np.concatenate([r["y"].reshape(BPC, P, H, W) for r in res.results], axis=0)
    return y, res


def kernel(x, w_high, w_low, act_scale_high, act_scale_low):
    y, _ = _run(x, w_high, w_low, act_scale_high, act_scale_low)
    return y


# revision 18
# speedup vs baseline: 1.0286x; 1.0286x over previous
"""DRQConv2d (dual-region quantized conv) Trainium2 kernel, v4.

Reference semantics:
  mask  = upsample8(avgpool8(x) >= 0.05)             per (b, c)
  xh    = where(mask, x, 1e-5);  xl = where(mask, 1e-5, x)
  qh    = clip(round(xh/sh), 0, 255) * sh            (uint8 fake-quant)
  ql    = clip(round(xl/sl), 0, 15) * sl             (uint4 fake-quant)
  qwh   = per-oc quant of w_high to +-127,  qwl = per-oc quant of w_low to +-7
  y     = conv3x3(qh, qwh) + conv3x3(ql, qwl)        (pad 1)

v4 design (v3 112us -> target ~85us):
  * Acts stored in zero-padded [58 x 58] planes so every tap runs the full
    8x56 output region: no border fixups, no mini matmuls.
    14 matmuls/chunk: 9 bf16 high taps + 4 fp8 DoubleRow low tap-pairs
    ((0,k)|(2,k) vertical, (1,0)|(1,2) horizontal) + 1 fp8 single (1,1).
  * High weights pre-scaled by ratio_oc = svh/svl in bf16 so both convs
    share one PSUM bank and a single sv_l evacuation scale.
    Low weights are exact ints in fp8 e4m3.
  * No explicit integer rounding: acts are bf16(relu(x/sh)) -> min/mask;
    the bf16 grid (exact ints to 256) is the high quantizer and the fp8
    RTN is the low quantizer (within the 2e-2 tolerance; TRN fp8e4 is
    IEEE e4m3, max normal 240).
  * Low-path act plane derived from the high-path relu for free:
    ql = min(t_h, 15*sl/sh) * (mask ? 0 : sh/sl)  -- the scale ratio is
    folded into the STT clip constant and the mask values.
  * Mask pipeline: DVE tensor_reduce (8-col then 8-row sums), GPSIMD
    threshold + mask-value TS + two small broadcast expansions.
  * One big 4D STT per path per image quantizes the whole padded plane.
  * Evacuation: ACT per-oc scale by svl, then DMA out.

Sharding: data-parallel over batch.  32 images -> 4 per core on 8 cores,
weights replicated; outputs concatenated on host.  No collectives.
"""

import numpy as np
import ml_dtypes

P = 128            # channels (both in and out) == partitions
B_TOTAL = 32
N_CORES = 8
BPC = B_TOTAL // N_CORES   # images per core
H = W = 56
NPIX = H * W       # 3136
NTAPS = 9
RPC = 8                       # output rows per chunk
NCHUNK = H // RPC             # 7
NFREE = RPC * W               # 448 psum columns per chunk
PW = W + 2                    # padded row length (58)
PH = H + 2
PADPIX = PH * PW              # 3364
POOL_K = 8
THRESH = 0.05
WARM_MMS = 20
# low-conv DR tap pairs (j0, j1) and the leftover single tap
LOW_PAIRS = (((0, 0), (2, 0)), ((0, 1), (2, 1)), ((0, 2), (2, 2)),
             ((1, 0), (1, 2)))
LOW_SINGLE = (1, 1)


# ---------------------------------------------------------------- host side

def _host_weight_prep(w, n):
    """Quantize per-oc exactly like the reference (fp32 divide + round-half-
    even + clip).  Returns integer weights [oc, ic, 9] (as fp32) and the
    per-oc weight scale s = absmax/n (fp32)."""
    w = np.asarray(w, dtype=np.float32).reshape(P, P, NTAPS)
    absmax = np.abs(w.reshape(P, -1)).max(axis=1).astype(np.float32)
    s = (absmax / np.float32(n)).astype(np.float32)
    ratio = w / s[:, None, None]          # fp32, like the reference
    wint = np.clip(np.round(ratio), -n, n).astype(np.float32)
    return wint, s


def _prep_inputs(w_high, w_low, act_scale_high, act_scale_low):
    sh = np.float64(np.float32(act_scale_high))
    sl = np.float64(np.float32(act_scale_low))
    inv_sh = float(np.float32(1.0 / sh))
    clip_lo = float(np.float32(15.0 * sl / sh))   # low clip in t_h units
    mlo = float(np.float32(sh / sl))              # low mask value

    wih, s_h = _host_weight_prep(w_high, 127.0)
    wil, s_l = _host_weight_prep(w_low, 7.0)

    bf16 = ml_dtypes.bfloat16
    e4 = ml_dtypes.float8_e4m3

    sv_h64 = sh * s_h.astype(np.float64)
    sv_l64 = sl * s_l.astype(np.float64)
    ratio = (sv_h64 / sv_l64)[None, None, :]      # per-oc

    # high: [ic, tap, oc] bf16, pre-scaled by svh/svl
    qwt_h = np.ascontiguousarray(
        (wih.transpose(1, 2, 0).astype(np.float64) * ratio)
        .astype(bf16)).reshape(P, NTAPS * P)
    # low DR pairs: [ic, pair, j, oc] fp8 (exact small ints)
    wil_t = wil.transpose(1, 2, 0)                # [ic, tap, oc]
    pairs = np.stack([
        np.stack([wil_t[:, a[0] * 3 + a[1], :], wil_t[:, b[0] * 3 + b[1], :]],
                 axis=1)
        for a, b in LOW_PAIRS], axis=1)           # [ic, pair, j, oc]
    qwt_l_p = np.ascontiguousarray(pairs.astype(e4)).reshape(P, 8 * P)
    # low single: [ic, oc] fp8
    ks, kws = LOW_SINGLE
    qwt_l_s = np.ascontiguousarray(
        wil_t[:, ks * 3 + kws, :].astype(e4)).reshape(P, P)

    return {
        "qwt_h": qwt_h,
        "qwt_l_p": qwt_l_p,
        "qwt_l_s": qwt_l_s,
        "sv_l": sv_l64.astype(np.float32).reshape(P, 1),
    }, inv_sh, clip_lo, mlo


# ---------------------------------------------------------------- device side

def build_program(nc, tc, aps, inv_sh, clip_lo, mlo, bpc=BPC):
    import concourse.mybir as mybir
    from concourse.alu_op_type import AluOpType as op

    f32 = mybir.dt.float32
    bf16 = mybir.dt.bfloat16
    fp8 = mybir.dt.float8e4
    DR = mybir.MatmulPerfMode.DoubleRow
    AX = mybir.AxisListType.X

    x_d, y_d = aps["x"], aps["y"]
    sum_thresh = float(np.float32(THRESH) * POOL_K * POOL_K)  # exact pow2 scale

    with (
        tc.tile_pool(name="consts", bufs=1) as consts,
        tc.tile_pool(name="xs", bufs=4) as xs_pool,
        tc.tile_pool(name="ts", bufs=3) as ts_pool,
        tc.tile_pool(name="qs", bufs=3) as qs_pool,
        tc.tile_pool(name="mk", bufs=2) as mk_pool,
        tc.tile_pool(name="ev", bufs=4) as ev_pool,
        tc.tile_pool(name="cps", bufs=8, space="PSUM") as cps,
    ):
        # ---- weights / scales (host-prepped, DMA only)
        qwt_h = consts.tile([P, NTAPS * P], bf16, tag="qwt_h")
        qwt_l_p = consts.tile([P, 8 * P], fp8, tag="qwt_l_p")
        qwt_l_s = consts.tile([P, P], fp8, tag="qwt_l_s")
        sv_l = consts.tile([P, 1], f32, tag="sv_l")

        # ---- PE warm-up: no data deps; runs while DMAs stream in.
        warm_l = consts.tile([P, P], bf16, tag="warm_l")
        warm_r = consts.tile([P, NFREE], bf16, tag="warm_r")
        nc.gpsimd.memset(warm_l[:], 0.0)
        nc.gpsimd.memset(warm_r[:], 0.0)
        warm_ps = cps.tile([P, NFREE], f32, tag="ps", name="warm")
        for i in range(WARM_MMS):
            nc.tensor.matmul(
                warm_ps[:], warm_l[:], warm_r[:],
                start=(i == 0), stop=(i == WARM_MMS - 1),
            )

        # ---- input DMAs; order sets HBM arrival priority for the head
        xts = {}
        for b in range(bpc):
            xts[b] = xs_pool.tile([P, NPIX], f32, tag="xt", name=f"xt{b}")

        def band_dma(b, row0, rows):
            nc.sync.dma_start(
                out=xts[b][:, row0 * W:(row0 + rows) * W],
                in_=x_d[b][:, row0 * W:(row0 + rows) * W],
            )

        band_dma(0, 0, 16)
        nc.sync.dma_start(out=qwt_h[:], in_=aps["qwt_h"])
        nc.sync.dma_start(out=qwt_l_p[:], in_=aps["qwt_l_p"])
        nc.sync.dma_start(out=qwt_l_s[:], in_=aps["qwt_l_s"])
        nc.sync.dma_start(out=sv_l[:], in_=aps["sv_l"])
        band_dma(0, 16, 16)
        band_dma(0, 32, 24)
        for b in range(1, bpc):
            band_dma(b, 0, 24)
            band_dma(b, 24, 32)

        # padded act planes; pad cells are zeroed once per physical buffer
        qh_t, ql_t = {}, {}
        for b in range(bpc):
            qh_t[b] = qs_pool.tile([P, PADPIX], bf16, tag="qh", name=f"qh{b}")
            ql_t[b] = qs_pool.tile([P, PADPIX], fp8, tag="ql", name=f"ql{b}")

        def pad_memsets(b):
            """Zero only the padding ring (rows 0/57, cols 0/57)."""
            for q in (qh_t[b], ql_t[b]):
                q3 = q[:].rearrange("p (r c) -> p r c", c=PW)
                nc.gpsimd.memset(q3[:, 0:PH:PH - 1, :], 0.0)
                nc.gpsimd.memset(q3[:, 1:PH - 1, 0:PW:PW - 1], 0.0)

        def image_tiles(b):
            t = ts_pool.tile([P, NPIX], bf16, tag="t", name=f"t{b}")
            p1 = mk_pool.tile([P, 392], f32, tag="p1", name=f"p1_{b}")
            u1 = mk_pool.tile([P, 196], f32, tag="u1", name=f"u1_{b}")
            u2 = mk_pool.tile([P, 98], f32, tag="u2", name=f"u2_{b}")
            s2 = mk_pool.tile([P, 49], f32, tag="s2", name=f"s2_{b}")
            mlw = mk_pool.tile([P, 49], f32, tag="mlw", name=f"mlw_{b}")
            mw_h = mk_pool.tile([P, 392], bf16, tag="mw_h", name=f"mwh_{b}")
            mw_l = mk_pool.tile([P, 392], bf16, tag="mw_l", name=f"mwl_{b}")
            return t, p1, u1, u2, s2, mlw, mw_h, mw_l

        prep = {}

        def prep_bands(b, hb0, nhb):
            """Prep pool-rows [hb0, hb0+nhb): relu, pool, mask, quantize.
            The small mask-chain ops run on DVE for image 0 (short critical
            path at the head) and on GPSIMD for the rest (keeps DVE free)."""
            t, p1, u1, u2, s2, mlw, mw_h, mw_l = prep[b]
            g = nc.vector if b == 0 else nc.gpsimd
            xt = xts[b]
            r0, rows = hb0 * POOL_K, nhb * POOL_K

            # ACT: t = bf16(relu(x * inv_sh))
            nc.scalar.activation(
                t[:, r0 * W:(r0 + rows) * W], xt[:, r0 * W:(r0 + rows) * W],
                mybir.ActivationFunctionType.Relu, scale=inv_sh,
            )
            # DVE: pool 8 cols: [P, rows*7, 8] -> [P, rows*7]  (row-major
            # (r, wb) fuses to a stride-8 dim)
            nc.vector.tensor_reduce(
                p1[:, r0 * 7:(r0 + rows) * 7],
                xt[:, r0 * W:(r0 + rows) * W].rearrange(
                    "p (k c) -> p k c", c=POOL_K),
                AX, op.add,
            )
            # pool 8 rows as a 3-round pairwise tree over (r, wb)
            sr0, scount = r0, rows
            for src, dst in ((p1, u1), (u1, u2), (u2, s2)):
                vin = src[:, sr0 * 7:(sr0 + scount) * 7].rearrange(
                    "p (r wb) -> p r wb", wb=7)
                g.tensor_tensor(
                    out=dst[:, (sr0 // 2) * 7:((sr0 + scount) // 2) * 7]
                    .rearrange("p (r wb) -> p r wb", wb=7),
                    in0=vin[:, 0:scount:2, :], in1=vin[:, 1:scount:2, :],
                    op=op.add)
                sr0, scount = sr0 // 2, scount // 2
            # threshold -> {0,1}; low mask value {mlo, 0}
            g.tensor_scalar(
                s2[:, hb0 * 7:(hb0 + nhb) * 7],
                s2[:, hb0 * 7:(hb0 + nhb) * 7],
                sum_thresh, None, op0=op.is_ge)
            g.tensor_scalar(
                mlw[:, hb0 * 7:(hb0 + nhb) * 7],
                s2[:, hb0 * 7:(hb0 + nhb) * 7],
                -mlo, mlo, op0=op.mult, op1=op.add)
            # DVE: expand wb -> 8 cols: [P, nhb*56] bf16 row patterns
            for src, dst in ((s2, mw_h), (mlw, mw_l)):
                nc.vector.tensor_copy(
                    out=dst[:, hb0 * 56:(hb0 + nhb) * 56].rearrange(
                        "p (wb c) -> p wb c", c=POOL_K),
                    in_=src[:, hb0 * 7:(hb0 + nhb) * 7].unsqueeze(2)
                    .broadcast_to((P, nhb * 7, POOL_K)),
                )
            # DVE: per-hb STT: clip + mask (8-row broadcast) -> padded plane
            t3 = t[:].rearrange("p (r c) -> p r c", c=W)
            qh3 = qh_t[b][:].rearrange("p (r c) -> p r c", c=PW)
            ql3 = ql_t[b][:].rearrange("p (r c) -> p r c", c=PW)
            for hb in range(hb0, hb0 + nhb):
                for q3, clip, mw in ((qh3, 255.0, mw_h),
                                     (ql3, clip_lo, mw_l)):
                    in1 = mw[:, hb * 56:(hb + 1) * 56].unsqueeze(1)
                    in1 = in1.broadcast_to((P, POOL_K, W))
                    nc.vector.scalar_tensor_tensor(
                        out=q3[:, 1 + hb * POOL_K:1 + (hb + 1) * POOL_K,
                               1:1 + W],
                        in0=t3[:, hb * POOL_K:(hb + 1) * POOL_K, :],
                        scalar=clip, in1=in1, op0=op.min, op1=op.mult,
                    )

        def conv_chunk(b, c):
            """9 bf16 high taps + 4 DR low pairs + 1 low single + evac."""
            qh3 = qh_t[b][:].rearrange("p (r c) -> p r c", c=PW)
            ql3 = ql_t[b][:].rearrange("p (r c) -> p r c", c=PW)
            r0 = c * RPC
            ps = cps.tile([P, NFREE], f32, tag="ps", name=f"ps{b}_{c}")
            for i, (kh, kw) in enumerate(
                    (kh, kw) for kh in range(3) for kw in range(3)):
                nc.tensor.matmul(
                    ps[:],
                    qwt_h[:, (kh * 3 + kw) * P:(kh * 3 + kw + 1) * P],
                    qh3[:, r0 + kh:r0 + kh + RPC, kw:kw + W],
                    start=(i == 0), stop=False,
                )
            for pi, (ta, tb) in enumerate(LOW_PAIRS):
                off_a = (r0 + ta[0]) * PW + ta[1]
                off_b = (r0 + tb[0]) * PW + tb[1]
                rhs = ql3[:, r0 + ta[0]:r0 + ta[0] + RPC, ta[1]:ta[1] + W]
                rhs = rhs.unsqueeze(1).broadcast_to((P, 2, RPC, W))
                rhs.ap = mybir.VecI64Pair(
                    [[PADPIX, P], [off_b - off_a, 2], [PW, RPC], [1, W]])
                nc.tensor.matmul(
                    ps[:],
                    qwt_l_p[:, pi * 2 * P:(pi + 1) * 2 * P]
                    .rearrange("p (j m) -> p j m", j=2),
                    rhs,
                    start=False, stop=False,
                    perf_mode=DR,
                )
            ks, kws = LOW_SINGLE
            nc.tensor.matmul(
                ps[:], qwt_l_s[:],
                ql3[:, r0 + ks:r0 + ks + RPC, kws:kws + W],
                start=False, stop=True,
            )
            acc = ev_pool.tile([P, NFREE], f32, tag="acc")
            nc.scalar.mul(acc[:], ps[:], sv_l[:, 0:1])
            nc.sync.dma_start(
                out=y_d[b][:, r0 * W:(r0 + RPC) * W], in_=acc[:],
            )

        # ---------------- schedule ----------------
        # image 0 in 3 bands to cut head latency
        pad_memsets(0)
        prep[0] = image_tiles(0)
        prep_bands(0, 0, 2)
        prep_bands(0, 2, 2)
        conv_chunk(0, 0)
        prep_bands(0, 4, 3)
        conv_chunk(0, 1)
        conv_chunk(0, 2)
        # image 1 prep next so in-order queues stay ahead of conv evacs
        if bpc > 1:
            pad_memsets(1)
            if bpc > 2:
                pad_memsets(2)
            prep[1] = image_tiles(1)
            prep_bands(1, 0, 7)
        for c in range(3, NCHUNK):
            conv_chunk(0, c)

        for b in range(1, bpc):
            nb = b + 1
            for c in range(NCHUNK):
                if nb < bpc and c == 1:
                    prep[nb] = image_tiles(nb)
                    prep_bands(nb, 0, 7)
                conv_chunk(b, c)


def make_bass(inv_sh, clip_lo, mlo, bpc=BPC):
    import concourse.bacc as bacc
    import concourse.mybir as mybir
    from concourse.tile import TileContext

    f32 = mybir.dt.float32
    bf16 = mybir.dt.bfloat16
    fp8 = mybir.dt.float8e4
    nc = bacc.Bacc("TRN2", debug=False)
    x = nc.dram_tensor("x", [bpc, P, NPIX], f32, kind="ExternalInput")
    qwh = nc.dram_tensor("qwt_h", [P, NTAPS * P], bf16, kind="ExternalInput")
    qwlp = nc.dram_tensor("qwt_l_p", [P, 8 * P], fp8, kind="ExternalInput")
    qwls = nc.dram_tensor("qwt_l_s", [P, P], fp8, kind="ExternalInput")
    svl = nc.dram_tensor("sv_l", [P, 1], f32, kind="ExternalInput")
    y = nc.dram_tensor("y", [bpc, P, NPIX], f32, kind="ExternalOutput")
    aps = {
        "x": x.ap(), "y": y.ap(),
        "qwt_h": qwh.ap(), "qwt_l_p": qwlp.ap(), "qwt_l_s": qwls.ap(),
        "sv_l": svl.ap(),
    }
    with TileContext(nc) as tc:
        build_program(nc, tc, aps, inv_sh, clip_lo, mlo, bpc=bpc)
    nc.compile()
    return nc


def _run(x, w_high, w_low, act_scale_high, act_scale_low, trace=False, **kw):
    from concourse import bass_utils

    x = np.ascontiguousarray(np.asarray(x, dtype=np.float32))
    w_high = np.asarray(w_high, dtype=np.float32)
    w_low = np.asarray(w_low, dtype=np.float32)

    wmap, inv_sh, clip_lo, mlo = _prep_inputs(
        w_high, w_low, act_scale_high, act_scale_low)
    nc = make_bass(inv_sh, clip_lo, mlo)

    in_maps = []
    for core in range(N_CORES):
        xs = x[core * BPC:(core + 1) * BPC].reshape(BPC, P, NPIX)
        m = {"x": np.ascontiguousarray(xs)}
        m.update(wmap)
        in_maps.append(m)
    res = bass_utils.run_bass_kernel_spmd(
        nc, in_maps, core_ids=list(range(N_CORES)), trace=trace, **kw
    )
    y = np.concatenate([r["y"].reshape(BPC, P, H, W) for r in res.results], axis=0)
    return y, res


def kernel(x, w_high, w_low, act_scale_high, act_scale_low):
    y, _ = _run(x, w_high, w_low, act_scale_high, act_scale_low)
    return y


# revision 19
# speedup vs baseline: 1.0376x; 1.0087x over previous
"""DRQConv2d (dual-region quantized conv) Trainium2 kernel, v4.

Reference semantics:
  mask  = upsample8(avgpool8(x) >= 0.05)             per (b, c)
  xh    = where(mask, x, 1e-5);  xl = where(mask, 1e-5, x)
  qh    = clip(round(xh/sh), 0, 255) * sh            (uint8 fake-quant)
  ql    = clip(round(xl/sl), 0, 15) * sl             (uint4 fake-quant)
  qwh   = per-oc quant of w_high to +-127,  qwl = per-oc quant of w_low to +-7
  y     = conv3x3(qh, qwh) + conv3x3(ql, qwl)        (pad 1)

v4 design (v3 112us -> target ~85us):
  * Acts stored in zero-padded [58 x 58] planes so every tap runs the full
    8x56 output region: no border fixups, no mini matmuls.
    14 matmuls/chunk: 9 bf16 high taps + 4 fp8 DoubleRow low tap-pairs
    ((0,k)|(2,k) vertical, (1,0)|(1,2) horizontal) + 1 fp8 single (1,1).
  * High weights pre-scaled by ratio_oc = svh/svl in bf16 so both convs
    share one PSUM bank and a single sv_l evacuation scale.
    Low weights are exact ints in fp8 e4m3.
  * No explicit integer rounding: acts are bf16(relu(x/sh)) -> min/mask;
    the bf16 grid (exact ints to 256) is the high quantizer and the fp8
    RTN is the low quantizer (within the 2e-2 tolerance; TRN fp8e4 is
    IEEE e4m3, max normal 240).
  * Low-path act plane derived from the high-path relu for free:
    ql = min(t_h, 15*sl/sh) * (mask ? 0 : sh/sl)  -- the scale ratio is
    folded into the STT clip constant and the mask values.
  * Mask pipeline: DVE tensor_reduce (8-col then 8-row sums), GPSIMD
    threshold + mask-value TS + two small broadcast expansions.
  * One big 4D STT per path per image quantizes the whole padded plane.
  * Evacuation: ACT per-oc scale by svl, then DMA out.

Sharding: data-parallel over batch.  32 images -> 4 per core on 8 cores,
weights replicated; outputs concatenated on host.  No collectives.
"""

import numpy as np
import ml_dtypes

P = 128            # channels (both in and out) == partitions
B_TOTAL = 32
N_CORES = 8
BPC = B_TOTAL // N_CORES   # images per core
H = W = 56
NPIX = H * W       # 3136
NTAPS = 9
RPC = 8                       # output rows per chunk
NCHUNK = H // RPC             # 7
NFREE = RPC * W               # 448 psum columns per chunk
PW = W + 2                    # padded row length (58)
PH = H + 2
PADPIX = PH * PW              # 3364
POOL_K = 8
THRESH = 0.05
WARM_MMS = 20
# low-conv DR tap pairs (j0, j1) and the leftover single tap
LOW_PAIRS = (((0, 0), (2, 0)), ((0, 1), (2, 1)), ((0, 2), (2, 2)),
             ((1, 0), (1, 2)))
LOW_SINGLE = (1, 1)


# ---------------------------------------------------------------- host side

def _host_weight_prep(w, n):
    """Quantize per-oc exactly like the reference (fp32 divide + round-half-
    even + clip).  Returns integer weights [oc, ic, 9] (as fp32) and the
    per-oc weight scale s = absmax/n (fp32)."""
    w = np.asarray(w, dtype=np.float32).reshape(P, P, NTAPS)
    absmax = np.abs(w.reshape(P, -1)).max(axis=1).astype(np.float32)
    s = (absmax / np.float32(n)).astype(np.float32)
    ratio = w / s[:, None, None]          # fp32, like the reference
    wint = np.clip(np.round(ratio), -n, n).astype(np.float32)
    return wint, s


def _prep_inputs(w_high, w_low, act_scale_high, act_scale_low):
    sh = np.float64(np.float32(act_scale_high))
    sl = np.float64(np.float32(act_scale_low))
    inv_sh = float(np.float32(1.0 / sh))
    clip_lo = float(np.float32(15.0 * sl / sh))   # low clip in t_h units
    mlo = float(np.float32(sh / sl))              # low mask value

    wih, s_h = _host_weight_prep(w_high, 127.0)
    wil, s_l = _host_weight_prep(w_low, 7.0)

    bf16 = ml_dtypes.bfloat16
    e4 = ml_dtypes.float8_e4m3

    sv_h64 = sh * s_h.astype(np.float64)
    sv_l64 = sl * s_l.astype(np.float64)
    ratio = (sv_h64 / sv_l64)[None, None, :]      # per-oc

    # high: [ic, tap, oc] bf16, pre-scaled by svh/svl
    qwt_h = np.ascontiguousarray(
        (wih.transpose(1, 2, 0).astype(np.float64) * ratio)
        .astype(bf16)).reshape(P, NTAPS * P)
    # low DR pairs: [ic, pair, j, oc] fp8 (exact small ints)
    wil_t = wil.transpose(1, 2, 0)                # [ic, tap, oc]
    pairs = np.stack([
        np.stack([wil_t[:, a[0] * 3 + a[1], :], wil_t[:, b[0] * 3 + b[1], :]],
                 axis=1)
        for a, b in LOW_PAIRS], axis=1)           # [ic, pair, j, oc]
    qwt_l_p = np.ascontiguousarray(pairs.astype(e4)).reshape(P, 8 * P)
    # low single: [ic, oc] fp8
    ks, kws = LOW_SINGLE
    qwt_l_s = np.ascontiguousarray(
        wil_t[:, ks * 3 + kws, :].astype(e4)).reshape(P, P)

    return {
        "qwt_h": qwt_h,
        "qwt_l_p": qwt_l_p,
        "qwt_l_s": qwt_l_s,
        "sv_l": sv_l64.astype(np.float32).reshape(P, 1),
    }, inv_sh, clip_lo, mlo


# ---------------------------------------------------------------- device side

def build_program(nc, tc, aps, inv_sh, clip_lo, mlo, bpc=BPC):
    import concourse.mybir as mybir
    from concourse.alu_op_type import AluOpType as op

    f32 = mybir.dt.float32
    bf16 = mybir.dt.bfloat16
    fp8 = mybir.dt.float8e4
    DR = mybir.MatmulPerfMode.DoubleRow
    AX = mybir.AxisListType.X

    x_d, y_d = aps["x"], aps["y"]
    sum_thresh = float(np.float32(THRESH) * POOL_K * POOL_K)  # exact pow2 scale

    with (
        tc.tile_pool(name="consts", bufs=1) as consts,
        tc.tile_pool(name="xs", bufs=4) as xs_pool,
        tc.tile_pool(name="ts", bufs=3) as ts_pool,
        tc.tile_pool(name="qs", bufs=3) as qs_pool,
        tc.tile_pool(name="mk", bufs=2) as mk_pool,
        tc.tile_pool(name="ev", bufs=4) as ev_pool,
        tc.tile_pool(name="cps", bufs=8, space="PSUM") as cps,
    ):
        # ---- weights / scales (host-prepped, DMA only)
        qwt_h = consts.tile([P, NTAPS * P], bf16, tag="qwt_h")
        qwt_l_p = consts.tile([P, 8 * P], fp8, tag="qwt_l_p")
        qwt_l_s = consts.tile([P, P], fp8, tag="qwt_l_s")
        sv_l = consts.tile([P, 1], f32, tag="sv_l")

        # ---- PE warm-up: no data deps; runs while DMAs stream in.
        warm_l = consts.tile([P, P], bf16, tag="warm_l")
        warm_r = consts.tile([P, NFREE], bf16, tag="warm_r")
        nc.gpsimd.memset(warm_l[:], 0.0)
        nc.gpsimd.memset(warm_r[:], 0.0)
        warm_ps = cps.tile([P, NFREE], f32, tag="ps", name="warm")
        for i in range(WARM_MMS):
            nc.tensor.matmul(
                warm_ps[:], warm_l[:], warm_r[:],
                start=(i == 0), stop=(i == WARM_MMS - 1),
            )

        # ---- input DMAs; order sets HBM arrival priority for the head
        xts = {}
        for b in range(bpc):
            xts[b] = xs_pool.tile([P, NPIX], f32, tag="xt", name=f"xt{b}")

        def band_dma(b, row0, rows):
            nc.sync.dma_start(
                out=xts[b][:, row0 * W:(row0 + rows) * W],
                in_=x_d[b][:, row0 * W:(row0 + rows) * W],
            )

        band_dma(0, 0, 16)
        nc.sync.dma_start(out=qwt_h[:], in_=aps["qwt_h"])
        nc.sync.dma_start(out=qwt_l_p[:], in_=aps["qwt_l_p"])
        nc.sync.dma_start(out=qwt_l_s[:], in_=aps["qwt_l_s"])
        nc.sync.dma_start(out=sv_l[:], in_=aps["sv_l"])
        band_dma(0, 16, 16)
        band_dma(0, 32, 24)
        for b in range(1, bpc):
            band_dma(b, 0, 24)
            band_dma(b, 24, 32)

        # padded act planes; pad cells are zeroed once per physical buffer
        qh_t, ql_t = {}, {}
        for b in range(bpc):
            qh_t[b] = qs_pool.tile([P, PADPIX], bf16, tag="qh", name=f"qh{b}")
            ql_t[b] = qs_pool.tile([P, PADPIX], fp8, tag="ql", name=f"ql{b}")

        def pad_memsets(b):
            """Zero only the padding ring (rows 0/57, cols 0/57)."""
            for q in (qh_t[b], ql_t[b]):
                q3 = q[:].rearrange("p (r c) -> p r c", c=PW)
                nc.gpsimd.memset(q3[:, 0:PH:PH - 1, :], 0.0)
                nc.gpsimd.memset(q3[:, 1:PH - 1, 0:PW:PW - 1], 0.0)

        def image_tiles(b):
            t = ts_pool.tile([P, NPIX], bf16, tag="t", name=f"t{b}")
            p1 = mk_pool.tile([P, 392], f32, tag="p1", name=f"p1_{b}")
            u1 = mk_pool.tile([P, 196], f32, tag="u1", name=f"u1_{b}")
            u2 = mk_pool.tile([P, 98], f32, tag="u2", name=f"u2_{b}")
            s2 = mk_pool.tile([P, 49], f32, tag="s2", name=f"s2_{b}")
            mlw = mk_pool.tile([P, 49], f32, tag="mlw", name=f"mlw_{b}")
            mw_h = mk_pool.tile([P, 392], bf16, tag="mw_h", name=f"mwh_{b}")
            mw_l = mk_pool.tile([P, 392], bf16, tag="mw_l", name=f"mwl_{b}")
            return t, p1, u1, u2, s2, mlw, mw_h, mw_l

        prep = {}

        def prep_bands(b, hb0, nhb):
            """Prep pool-rows [hb0, hb0+nhb): relu, pool, mask, quantize.
            The small mask-chain ops run on DVE for image 0 (short critical
            path at the head) and on GPSIMD for the rest (keeps DVE free)."""
            t, p1, u1, u2, s2, mlw, mw_h, mw_l = prep[b]
            g = nc.vector if (b == 0 and hb0 == 0) else nc.gpsimd
            xt = xts[b]
            r0, rows = hb0 * POOL_K, nhb * POOL_K

            # ACT: t = bf16(relu(x * inv_sh))
            nc.scalar.activation(
                t[:, r0 * W:(r0 + rows) * W], xt[:, r0 * W:(r0 + rows) * W],
                mybir.ActivationFunctionType.Relu, scale=inv_sh,
            )
            # DVE: pool 8 cols: [P, rows*7, 8] -> [P, rows*7]  (row-major
            # (r, wb) fuses to a stride-8 dim)
            nc.vector.tensor_reduce(
                p1[:, r0 * 7:(r0 + rows) * 7],
                xt[:, r0 * W:(r0 + rows) * W].rearrange(
                    "p (k c) -> p k c", c=POOL_K),
                AX, op.add,
            )
            # pool 8 rows as a 3-round pairwise tree over (r, wb)
            sr0, scount = r0, rows
            for src, dst in ((p1, u1), (u1, u2), (u2, s2)):
                vin = src[:, sr0 * 7:(sr0 + scount) * 7].rearrange(
                    "p (r wb) -> p r wb", wb=7)
                g.tensor_tensor(
                    out=dst[:, (sr0 // 2) * 7:((sr0 + scount) // 2) * 7]
                    .rearrange("p (r wb) -> p r wb", wb=7),
                    in0=vin[:, 0:scount:2, :], in1=vin[:, 1:scount:2, :],
                    op=op.add)
                sr0, scount = sr0 // 2, scount // 2
            # threshold -> {0,1}; low mask value {mlo, 0}
            g.tensor_scalar(
                s2[:, hb0 * 7:(hb0 + nhb) * 7],
                s2[:, hb0 * 7:(hb0 + nhb) * 7],
                sum_thresh, None, op0=op.is_ge)
            g.tensor_scalar(
                mlw[:, hb0 * 7:(hb0 + nhb) * 7],
                s2[:, hb0 * 7:(hb0 + nhb) * 7],
                -mlo, mlo, op0=op.mult, op1=op.add)
            # DVE: expand wb -> 8 cols: [P, nhb*56] bf16 row patterns
            for src, dst in ((s2, mw_h), (mlw, mw_l)):
                nc.vector.tensor_copy(
                    out=dst[:, hb0 * 56:(hb0 + nhb) * 56].rearrange(
                        "p (wb c) -> p wb c", c=POOL_K),
                    in_=src[:, hb0 * 7:(hb0 + nhb) * 7].unsqueeze(2)
                    .broadcast_to((P, nhb * 7, POOL_K)),
                )
            # DVE: per-hb STT: clip + mask (8-row broadcast) -> padded plane
            t3 = t[:].rearrange("p (r c) -> p r c", c=W)
            qh3 = qh_t[b][:].rearrange("p (r c) -> p r c", c=PW)
            ql3 = ql_t[b][:].rearrange("p (r c) -> p r c", c=PW)
            for hb in range(hb0, hb0 + nhb):
                for q3, clip, mw in ((qh3, 255.0, mw_h),
                                     (ql3, clip_lo, mw_l)):
                    in1 = mw[:, hb * 56:(hb + 1) * 56].unsqueeze(1)
                    in1 = in1.broadcast_to((P, POOL_K, W))
                    nc.vector.scalar_tensor_tensor(
                        out=q3[:, 1 + hb * POOL_K:1 + (hb + 1) * POOL_K,
                               1:1 + W],
                        in0=t3[:, hb * POOL_K:(hb + 1) * POOL_K, :],
                        scalar=clip, in1=in1, op0=op.min, op1=op.mult,
                    )

        def conv_chunk(b, c):
            """9 bf16 high taps + 4 DR low pairs + 1 low single + evac."""
            qh3 = qh_t[b][:].rearrange("p (r c) -> p r c", c=PW)
            ql3 = ql_t[b][:].rearrange("p (r c) -> p r c", c=PW)
            r0 = c * RPC
            ps = cps.tile([P, NFREE], f32, tag="ps", name=f"ps{b}_{c}")
            for i, (kh, kw) in enumerate(
                    (kh, kw) for kh in range(3) for kw in range(3)):
                nc.tensor.matmul(
                    ps[:],
                    qwt_h[:, (kh * 3 + kw) * P:(kh * 3 + kw + 1) * P],
                    qh3[:, r0 + kh:r0 + kh + RPC, kw:kw + W],
                    start=(i == 0), stop=False,
                )
            for pi, (ta, tb) in enumerate(LOW_PAIRS):
                off_a = (r0 + ta[0]) * PW + ta[1]
                off_b = (r0 + tb[0]) * PW + tb[1]
                rhs = ql3[:, r0 + ta[0]:r0 + ta[0] + RPC, ta[1]:ta[1] + W]
                rhs = rhs.unsqueeze(1).broadcast_to((P, 2, RPC, W))
                rhs.ap = mybir.VecI64Pair(
                    [[PADPIX, P], [off_b - off_a, 2], [PW, RPC], [1, W]])
                nc.tensor.matmul(
                    ps[:],
                    qwt_l_p[:, pi * 2 * P:(pi + 1) * 2 * P]
                    .rearrange("p (j m) -> p j m", j=2),
                    rhs,
                    start=False, stop=False,
                    perf_mode=DR,
                )
            ks, kws = LOW_SINGLE
            nc.tensor.matmul(
                ps[:], qwt_l_s[:],
                ql3[:, r0 + ks:r0 + ks + RPC, kws:kws + W],
                start=False, stop=True,
            )
            acc = ev_pool.tile([P, NFREE], f32, tag="acc")
            nc.scalar.mul(acc[:], ps[:], sv_l[:, 0:1])
            nc.sync.dma_start(
                out=y_d[b][:, r0 * W:(r0 + RPC) * W], in_=acc[:],
            )

        # ---------------- schedule ----------------
        # image 0 in 3 bands to cut head latency
        pad_memsets(0)
        prep[0] = image_tiles(0)
        prep_bands(0, 0, 2)
        prep_bands(0, 2, 2)
        conv_chunk(0, 0)
        prep_bands(0, 4, 3)
        conv_chunk(0, 1)
        conv_chunk(0, 2)
        # image 1 prep next so in-order queues stay ahead of conv evacs
        if bpc > 1:
            pad_memsets(1)
            if bpc > 2:
                pad_memsets(2)
            prep[1] = image_tiles(1)
            prep_bands(1, 0, 7)
        for c in range(3, NCHUNK):
            conv_chunk(0, c)

        for b in range(1, bpc):
            nb = b + 1
            for c in range(NCHUNK):
                if nb < bpc and c == 1:
                    prep[nb] = image_tiles(nb)
                    prep_bands(nb, 0, 7)
                conv_chunk(b, c)


def make_bass(inv_sh, clip_lo, mlo, bpc=BPC):
    import concourse.bacc as bacc
    import concourse.mybir as mybir
    from concourse.tile import TileContext

    f32 = mybir.dt.float32
    bf16 = mybir.dt.bfloat16
    fp8 = mybir.dt.float8e4
    nc = bacc.Bacc("TRN2", debug=False)
    x = nc.dram_tensor("x", [bpc, P, NPIX], f32, kind="ExternalInput")
    qwh = nc.dram_tensor("qwt_h", [P, NTAPS * P], bf16, kind="ExternalInput")
    qwlp = nc.dram_tensor("qwt_l_p", [P, 8 * P], fp8, kind="ExternalInput")
    qwls = nc.dram_tensor("qwt_l_s", [P, P], fp8, kind="ExternalInput")
    svl = nc.dram_tensor("sv_l", [P, 1], f32, kind="ExternalInput")
    y = nc.dram_tensor("y", [bpc, P, NPIX], f32, kind="ExternalOutput")
    aps = {
        "x": x.ap(), "y": y.ap(),
        "qwt_h": qwh.ap(), "qwt_l_p": qwlp.ap(), "qwt_l_s": qwls.ap(),
        "sv_l": svl.ap(),
    }
    with TileContext(nc) as tc:
        build_program(nc, tc, aps, inv_sh, clip_lo, mlo, bpc=bpc)
    nc.compile()
    return nc


def _run(x, w_high, w_low, act_scale_high, act_scale_low, trace=False, **kw):
    from concourse import bass_utils

    x = np.ascontiguousarray(np.asarray(x, dtype=np.float32))
    w_high = np.asarray(w_high, dtype=np.float32)
    w_low = np.asarray(w_low, dtype=np.float32)

    wmap, inv_sh, clip_lo, mlo = _prep_inputs(
        w_high, w_low, act_scale_high, act_scale_low)
    nc = make_bass(inv_sh, clip_lo, mlo)

    in_maps = []
    for core in range(N_CORES):
        xs = x[core * BPC:(core + 1) * BPC].reshape(BPC, P, NPIX)
        m = {"x": np.ascontiguousarray(xs)}
        m.update(wmap)
        in_maps.append(m)
    res = bass_utils.run_bass_kernel_spmd(
        nc, in_maps, core_ids=list(range(N_CORES)), trace=trace, **kw
    )
    y = np.concatenate([r["y"].reshape(BPC, P, H, W) for r in res.results], axis=0)
    return y, res


def kernel(x, w_high, w_low, act_scale_high, act_scale_low):
    y, _ = _run(x, w_high, w_low, act_scale_high, act_scale_low)
    return y
